# revision 1
# baseline (speedup 1.0000x reference)
"""CovPool kernel for 8 TRN2 NeuronCores.

reference semantics (B=32, N=16384, D=64):
    cov_b = (X_b - mean_b)^T (X_b - mean_b) / (N-1) + lam*I        (64x64)
    out   = sort(concat_b triu(cov_b)) reshaped to (B, 2080)

Device strategy (data parallel over batch, core c owns batches [4c, 4c+4)):
  - stream the 16 MB slab via gpsimd SWDGE (only path that sustains
    ~460 GB/s/core here; HWDGE tops out ~330) into resident SBUF chunks.
    Chunk schedule is small at the head (first cast starts early) and
    small at the tail (tiny final MM+dump tail).
  - cast fp32 -> bf16 split across scalar (ACT) and vector (DVE) engines
    into a pair-grouped layout: 129-col groups [slice_2g|slice_2g+1|ones].
    (gpsimd's Q7 copy is ~4x too slow; it only issues the stream DMAs.)
  - pair-packed Gram matmuls: lhsT = 128-col bf16 pair, rhs = 129 cols
    (the ones col accumulates per-half column sums), 64 MMs per batch
    accumulating into one (128,129) PSUM region:
        psum = [[ G_ee, junk, s_e ], [ junk, G_oo, s_o ]]
  - per batch: DVE copies PSUM -> SBUF, DMA to HBM (BPC,128,129).
  - host folds G = G_ee + G_oo, s = s_e + s_o, applies the rank-1 mean
    correction + lam*I, extracts triu, global sort (tiny O(B*D^2) work,
    same bucket as the host-side torch.unique merge-sort).
"""

import sys

sys.path.insert(0, "/opt/trn_rl_repo")

import numpy as np

from concourse import bacc, mybir
from concourse.tile import TileContext
from concourse.bass_utils import run_bass_kernel_spmd

B, N, D = 32, 16384, 64
NCORES = 8
BPC = B // NCORES  # batches per core
LAMBDA = 0.01
D_OUT = D * (D + 1) // 2  # 2080

CS = 2 * D + 1  # 129: pair + ones column
import os as _os0
DR = _os0.environ.get("COV_DR") == "1"  # fp8 DoubleRow: 2 row-blocks/MM
GS = 144 if DR else CS  # group stride; DR pads to satisfy step%16==0

# chunk schedule: (batch, row0, nrows) per core. nrows % 256 == 0 so each
# chunk is a whole number of slice pairs. Head and tail chunks are small
# to shrink the pipeline ramp (first cast waits on the first chunk) and
# the drain tail (last chunk's casts+MMs+dump are fully exposed).
import os as _os

CHUNKS = []
if _os.environ.get("COV_MID4K") == "1":
    _PAT_FIRST = [1024, 3072, 4096, 4096, 4096]
    _PAT_MID = [4096, 4096, 4096, 4096]
    _PAT_LAST = [4096, 4096, 4096, 2048, 1024, 1024]
else:
    _PAT_FIRST = [1024, 7168, 8192]
    _PAT_MID = [8192, 8192]
    _PAT_LAST = [8192, 6144, 1024, 1024]
for _b in range(BPC):
    pat = _PAT_FIRST if _b == 0 else (_PAT_LAST if _b == BPC - 1
                                      else _PAT_MID)
    _r0 = 0
    for _nr in pat:
        CHUNKS.append((_b, _r0, _nr))
        _r0 += _nr
    assert _r0 == N
NCHUNKS = len(CHUNKS)

f32 = mybir.dt.float32
# fp8e4 halves the PE's contended SBUF read bytes (LDW + moving) at the
# cost of ~3e-3 cov rel err (tolerance 2e-2); env-gated, default bf16
bf16 = (mybir.dt.float8e4 if (DR or _os.environ.get("COV_FP8") == "1")
        else mybir.dt.bfloat16)


def _chunk_groups(nrows):
    """(pairs, act_groups) for a chunk: ACT takes ~37.5% (it is ~1.7x
    slower per element than DVE)."""
    pairs = nrows // 256
    if DR:
        ga = max(2, 2 * round(pairs * 0.1875))
    else:
        ga = max(1, round(pairs * 0.375)) if pairs > 1 else 0
    return pairs, ga


def _dma_engines(nc, dma_eng):
    if dma_eng == "hw2":
        return [nc.sync, nc.scalar]
    if dma_eng == "gp":
        return [nc.gpsimd]
    if dma_eng == "sync":
        return [nc.sync]
    raise ValueError(dma_eng)


def _emit_body(nc, x, out, bufs, bbAs, bbBs, dumps, psum_pool, variant,
               dma_eng="gp"):
    engs = _dma_engines(nc, dma_eng)
    xf = x.rearrange("b n d -> b (n d)")
    # prologue: issue every chunk DMA up front
    if not variant.startswith("mm_"):
        for k, (b, r0, nr) in enumerate(CHUNKS):
            # first (small) chunk via HWDGE: ~0.6 us first-byte vs the
            # ~2.4 us SWDGE emission ramp, so the first cast starts early
            eng = nc.sync if k == 0 else engs[k % len(engs)]
            eng.dma_start(
                bufs[k][:],
                xf[b, r0 * D:(r0 + nr) * D]
                .rearrange("(p f) -> p f", p=128),
            )
    if variant == "dma_only":
        scrap = dumps[0]
        for k in range(NCHUNKS):
            nc.vector.tensor_reduce(
                out=scrap[:, 0:1], in_=bufs[k][:],
                axis=mybir.AxisListType.X, op=mybir.AluOpType.max,
            )
        for b in range(BPC):
            nc.sync.dma_start(
                out[b].rearrange("t p c -> p t c"),
                bufs[2][:, 0:2 * CS].rearrange("p (t c) -> p t c", c=CS))
        return

    do_cast = variant not in ("mm_nocast",)
    do_mm = variant not in ("castdma_only",)
    mm_i = {b: 0 for b in range(BPC)}
    psums = {}
    for k, (b, r0, nr) in enumerate(CHUNKS):
        if do_mm and b not in psums:
            # two accumulators per batch on different PSUM banks so
            # back-to-back MMs alternate banks (hides accumulate
            # turnaround); host folds the two halves like TL+BR
            psums[b] = [
                psum_pool.tile([128, CS], f32, tag=f"acc{b}_{t}",
                               name=f"acc{b}_{t}") for t in range(2)
            ]
        psum = psums.get(b)
        buf, bbA, bbB = bufs[k], bbAs[k], bbBs[k]
        pairs, ga = _chunk_groups(nr)
        gb = pairs - ga
        ha = ga * 2 * D
        vB = bbB[:].rearrange("p (g c) -> p g c", c=GS)
        inB = buf[:, ha:pairs * 2 * D].rearrange("p (g c) -> p g c",
                                                 c=2 * D)
        if do_cast:
            if ga > 0:
                vA = bbA[:].rearrange("p (g c) -> p g c", c=GS)
                inA = buf[:, 0:ha].rearrange("p (g c) -> p g c", c=2 * D)
                sa = max(1, ga // 2)
                nc.scalar.copy(vA[:, 0:sa, 0:2 * D], inA[:, 0:sa, :])
                if sa < ga:
                    nc.scalar.copy(vA[:, sa:ga, 0:2 * D],
                                   inA[:, sa:ga, :])
            sb = max(1, gb // 2)
            nc.vector.tensor_copy(vB[:, 0:sb, 0:2 * D], inB[:, 0:sb, :])
            if sb < gb:
                nc.vector.tensor_copy(vB[:, sb:gb, 0:2 * D],
                                      inB[:, sb:gb, :])
        if not do_mm:
            continue
        per_mm = 2 if DR else 1
        half_total = N // 256 // per_mm // 2  # MMs/accumulator/batch
        for h, bb, ng in ((0, bbA, ga), (1, bbB, gb)):
            bv = bb[:].rearrange("p (t c) -> p t c", c=GS)
            for q in range(ng // per_mm):
                i = mm_i[b]
                mm_i[b] += 1
                ps = psum[i % 2]
                j = i // 2
                if DR:
                    nc.tensor.matmul(
                        ps[:], bv[:, 2 * q:2 * q + 2, 0:2 * D],
                        bv[:, 2 * q:2 * q + 2, 0:CS],
                        start=(j == 0), stop=(j == half_total - 1),
                        perf_mode=mybir.MatmulPerfMode.DoubleRow,
                    )
                else:
                    nc.tensor.matmul(
                        ps[:], bb[:, q * CS:q * CS + 2 * D],
                        bb[:, q * CS:q * CS + CS],
                        start=(j == 0), stop=(j == half_total - 1),
                    )
        if mm_i[b] == N // 256 // per_mm and do_mm:
            dump = dumps[b % 2]
            nc.vector.tensor_copy(dump[:, 0:CS], psum[0][:])
            nc.vector.tensor_copy(dump[:, CS:2 * CS], psum[1][:])
            nc.sync.dma_start(
                out[b].rearrange("t p c -> p t c"),
                dump[:].rearrange("p (t c) -> p t c", c=CS))
    if not do_mm:
        for b in range(BPC):
            nc.sync.dma_start(
                out[b].rearrange("t p c -> p t c"),
                bufs[2][:, 0:2 * CS].rearrange("p (t c) -> p t c", c=CS))


def build_cov_kernel(bench_reps=None, variant="full", dma_eng="gp",
                     unroll=1):
    nc = bacc.Bacc("TRN2", target_bir_lowering=False, debug=False,
                   num_devices=NCORES)
    x = nc.dram_tensor("x", [BPC, N, D], f32, kind="ExternalInput")
    out = nc.dram_tensor("out", [BPC, 2, 128, CS], f32,
                         kind="ExternalOutput")

    with TileContext(nc) as tc:
        with (
            tc.tile_pool(name="stream", bufs=1) as sp,
            tc.tile_pool(name="work", bufs=1) as wp,
            tc.tile_pool(name="psum", bufs=1, space="PSUM") as pp,
        ):
            bufs, bbAs, bbBs = [], [], []
            for k, (b, r0, nr) in enumerate(CHUNKS):
                pairs, ga = _chunk_groups(nr)
                gb = pairs - ga
                bufs.append(sp.tile([128, nr * D // 128], f32,
                                    tag=f"ch{k}", name=f"ch{k}"))
                bbAs.append(sp.tile([128, max(1, ga) * GS], bf16,
                                    tag=f"bbA{k}", name=f"bbA{k}"))
                bbBs.append(sp.tile([128, gb * GS], bf16,
                                    tag=f"bbB{k}", name=f"bbB{k}"))
            dumps = [wp.tile([128, 2 * CS], f32, tag=f"dump{i}",
                             name=f"dump{i}") for i in range(2)]
            for t in bbAs + bbBs:
                # only the ones COLUMNS (129th of each group) need init;
                # casts overwrite the data cols and never touch these
                v = t[:].rearrange("p (g c) -> p g c", c=GS)
                nc.vector.memset(v[:, :, 2 * D:2 * D + 1], 1.0)
            if variant.startswith("mm_"):
                for t in bufs:
                    nc.vector.memset(t[:], 0.5)

            def body():
                for _ in range(unroll):
                    _emit_body(nc, x, out, bufs, bbAs, bbBs, dumps, pp,
                               variant, dma_eng)

            if bench_reps is None:
                body()
            else:
                with tc.For_i(0, bench_reps, 1):
                    body()

    nc.compile()
    return nc


_NC_CACHE = {}


def _get_kernel():
    if "nc" not in _NC_CACHE:
        _NC_CACHE["nc"] = build_cov_kernel()
    return _NC_CACHE["nc"]


def _in_maps(x_full: np.ndarray):
    return [
        {"x": np.ascontiguousarray(x_full[c * BPC:(c + 1) * BPC])}
        for c in range(NCORES)
    ]


def run_device(x_full: np.ndarray):
    """Run the bass kernel on 8 cores; returns per-core psum dumps,
    list of (BPC, 128, 129)."""
    nc = _get_kernel()
    res = run_bass_kernel_spmd(nc, _in_maps(x_full),
                               core_ids=list(range(NCORES)))
    return [res.results[c]["out"] for c in range(NCORES)]


def _assemble(ps: np.ndarray) -> np.ndarray:
    """(B, 2, 128, 129) psum dumps -> (B, 64, 64) covariance matrices.
    Axis 1 is the two interleaved PSUM accumulators; within each, rows
    0:64 / 64:128 are the even/odd slice Gram blocks and col 128 holds
    the per-half column sums."""
    p = ps.sum(axis=1)
    G = p[:, 0:D, 0:D] + p[:, D:2 * D, D:2 * D]
    s = p[:, 0:D, 2 * D] + p[:, D:2 * D, 2 * D]
    cov = (G - s[:, :, None] * s[:, None, :] / N) / (N - 1)
    cov += LAMBDA * np.eye(D, dtype=np.float32)
    return cov


def kernel(x: np.ndarray) -> np.ndarray:
    x = np.asarray(x, dtype=np.float32)
    ps = np.concatenate(run_device(x), axis=0)  # (B, 128, 129)
    cov = _assemble(ps)
    iu, ju = np.triu_indices(D)
    tri = cov[:, iu, ju]  # (B, D_OUT)
    return np.sort(tri.reshape(-1)).reshape(B, D_OUT).astype(np.float32)


if __name__ == "__main__":
    rng = np.random.default_rng(0)
    xt = rng.standard_normal((B, N, D), dtype=np.float32)
    o = kernel(xt)
    print("kernel out shape:", o.shape, o.dtype)

